# revision 12
# baseline (speedup 1.0000x reference)
"""Trainium2 Bass kernel for the span-scoring + top-k module.

Problem (see reference): score all (word, width) candidate spans of a
4096-word document with a 1-hidden-layer MLP over concatenated endpoint
embeddings + width embedding + doc-type embedding, add a width-prior
score, mask invalid spans, then take the global top-k over the flattened
(span, type) score matrix.

Key algebraic restructuring (exact up to fp32 rounding):
  span_embs @ W1 decomposes into
      As[w] + Ae[w+j] + (Aw[j] + dt + b1)
  where As = E @ W1[:1024], Ae = E @ W1[1024:2048] are per-WORD matmuls
  (20x fewer FLOPs than the naive per-span matmul), and the width/doc
  contributions are tiny per-width bias vectors.  For a fixed width j the
  end-gather Ae[w+j] is just a shifted slice - no gathers anywhere.

Sharding: words are split 8 ways (512 words/core + 19-word halo for the
end embeddings).  Weights are replicated.  Each core computes its
[512 words x 20 widths x 38 types] logit block; the host merges the
per-shard results and does the exact top-k (selection on 9.8K censored
candidates is host-side bookkeeping; all heavy compute is on-device).

Device layout per core (all fp32):
  ET   [8kt][128, 544]  E^T shard slice (host-transposed, zero-padded)
  W1 m-tiles streamed   [128, 8kt*128] (host-tiled so lhsT slices are
                        contiguous), m 0-7 -> As, 8-15 -> Ae
  As   [128, 8, 512], Ae [128, 8, 544]   (hidden on partitions)
  H_j = relu(As + Ae[:, :, j:j+512] + ctab[:, kt, j])   (DVE add + ACT relu)
  L_j = W2^T @ H_j  (+ validity mask via a K=1 ones-matmul into the same
        PSUM accumulation, + per-type bias b2s[:, j] on the PSUM->SBUF copy)
  out  lg [20, 38, 512] -> HBM
"""

import numpy as np

import concourse.bacc as bacc
import concourse.bass as bass
import concourse.mybir as mybir
import concourse.tile as tile
from concourse.bass_utils import run_bass_kernel_spmd

F32 = mybir.dt.float32
F16 = mybir.dt.float16
AF = mybir.ActivationFunctionType
ALU = mybir.AluOpType

NCORES = 8
NW = 4096          # num words
HS = 1024          # hidden size of encoder states
MAXW = 20          # max span width
EMB = 20
MLP = 1000         # mlp hidden
MLPP = 1024        # padded mlp hidden
NT = 38            # n types
NEG = -1e10
WC = 512           # words per core
WH = 544           # words + halo pad (>= 512 + 19, mult of 32)
KT = 8             # k tiles over HS
MT = 16            # m tiles over 2*MLPP
P = 128

_CACHE = {}

import os
FP32R_L1 = os.environ.get("FP32R_L1", "0") == "1"
FP32R_L2 = os.environ.get("FP32R_L2", "0") == "1"
F32R = mybir.dt.float32r


def _mm_l1(ap):
    return ap.bitcast(F32R) if FP32R_L1 else ap


def _mm_l2(ap):
    return ap.bitcast(F32R) if FP32R_L2 else ap


def _build_nc():
    """Build + compile the single-core SPMD Bass program.

    L1 runs in fp16 hi/lo split form: E = Eh + 2^-11*El, W1 = Wh + 2^-11*Wl
    (fp16 products are exact in the fp32 PSUM accumulator, so
    As = Eh@Wh + 2^-11*(Eh@Wl + El@Wh) is MORE accurate than a plain fp32
    matmul and streams 3 cycles/row instead of fp32's 4).
    """
    nc = bacc.Bacc(None, target_bir_lowering=False)

    eh_d = nc.dram_tensor("eh", [KT, P, WH], F16, kind="ExternalInput")
    el_d = nc.dram_tensor("el", [KT, P, WH], F16, kind="ExternalInput")
    wh_d = nc.dram_tensor("wh", [MT, P, KT * P], F16, kind="ExternalInput")
    wl_d = nc.dram_tensor("wl", [MT, P, KT * P], F16, kind="ExternalInput")
    w2_d = nc.dram_tensor("w2", [P, KT * NT], F32, kind="ExternalInput")
    ct_d = nc.dram_tensor("ct", [P, KT * MAXW], F32, kind="ExternalInput")
    mk_d = nc.dram_tensor("mskr", [MAXW, NT, WC], F32, kind="ExternalInput")
    lg_d = nc.dram_tensor("lg", [MAXW, NT, WC], F32, kind="ExternalOutput")

    LOSC = float(2.0 ** -11)

    with tile.TileContext(nc) as tc:
        with (
            tc.tile_pool(name="const", bufs=1) as constp,
            tc.tile_pool(name="acc", bufs=1) as accp,
            tc.tile_pool(name="w1p", bufs=3) as w1p,
            tc.tile_pool(name="mkp", bufs=4) as mkp,
            tc.tile_pool(name="hp", bufs=2) as hp,
            tc.tile_pool(name="lgp", bufs=4) as lgp,
            tc.tile_pool(name="psa", bufs=2, space="PSUM") as psa,
            tc.tile_pool(name="psb", bufs=1, space="PSUM") as psb,
            tc.tile_pool(name="psl", bufs=3, space="PSUM") as psl,
        ):
            # ---- constants / persistent tensors ----
            # first m-pair's weights go first so the PE can start ASAP
            w1h0 = w1p.tile([P, KT * P], F16, tag="w1h")
            w1l0 = w1p.tile([P, KT * P], F16, tag="w1l")
            nc.sync.dma_start(out=w1h0[:], in_=wh_d[0])
            nc.sync.dma_start(out=w1l0[:], in_=wl_d[0])
            eh = [constp.tile([P, WH], F16, tag=f"eh{kt}", name=f"eh{kt}")
                  for kt in range(KT)]
            el = [constp.tile([P, WH], F16, tag=f"el{kt}", name=f"el{kt}")
                  for kt in range(KT)]
            for kt in range(KT):
                nc.sync.dma_start(out=eh[kt][:], in_=eh_d[kt])
                nc.sync.dma_start(out=el[kt][:], in_=el_d[kt])
            w2 = constp.tile([P, KT * NT], F32)
            nc.sync.dma_start(out=w2[:], in_=w2_d[:])
            ct = constp.tile([P, KT * MAXW], F32)
            nc.sync.dma_start(out=ct[:], in_=ct_d[:])

            As = accp.tile([P, KT, WC], F32)
            Ae = accp.tile([P, KT, WH], F32)

            # ---- phase A: As/Ae = E @ W1{s,e}, fp16 hi/lo split ----
            # m-order pairs (kt_h, kt_h+8) so each hidden k-tile of As AND Ae
            # completes early, letting phase-B H adds overlap the tail.
            for mi in range(MT):
                kt_h, half = mi // 2, mi % 2
                m = kt_h + 8 * half  # 0,8,1,9,...  (m<8 -> As, m>=8 -> Ae)
                if mi == 0:
                    w1h, w1l = w1h0, w1l0
                else:
                    w1h = w1p.tile([P, KT * P], F16, tag="w1h")
                    w1l = w1p.tile([P, KT * P], F16, tag="w1l")
                    nc.sync.dma_start(out=w1h[:], in_=wh_d[m])
                    nc.sync.dma_start(out=w1l[:], in_=wl_d[m])
                dst, ncols = (As, WC) if m < KT else (Ae, WH)
                for c0, cn, pool, tagsuf in ((0, WC, psa, "a"), (WC, WH, psb, "b")):
                    if c0 >= ncols:
                        continue
                    if tagsuf == "b":
                        pb2 = pool.tile([P, 2 * (cn - c0)], F32, tag="phlb",
                                        name="phlb")
                        ph = pb2[:, 0:cn - c0]
                        pl = pb2[:, cn - c0:]
                    else:
                        ph = pool.tile([P, cn - c0], F32, tag="ph" + tagsuf,
                                       name="ph" + tagsuf)
                        pl = pool.tile([P, cn - c0], F32, tag="pl" + tagsuf,
                                       name="pl" + tagsuf)
                    for kt in range(KT):
                        nc.tensor.matmul(
                            ph[:], w1h[:, kt * P:(kt + 1) * P],
                            eh[kt][:, c0:cn],
                            start=(kt == 0), stop=(kt == KT - 1))
                    for kt in range(KT):
                        nc.tensor.matmul(
                            pl[:], w1l[:, kt * P:(kt + 1) * P],
                            eh[kt][:, c0:cn],
                            start=(kt == 0), stop=False)
                        nc.tensor.matmul(
                            pl[:], w1h[:, kt * P:(kt + 1) * P],
                            el[kt][:, c0:cn],
                            start=False, stop=(kt == KT - 1))
                    out_ap = dst[:, kt_h, c0:cn]
                    tmp = w1p.tile([P, WC], F32, tag="cmb", name="cmb")
                    nc.scalar.activation(tmp[:, 0:cn - c0], pl[:], AF.Copy,
                                         bias=0.0, scale=LOSC)
                    nc.vector.tensor_tensor(out_ap, ph[:], tmp[:, 0:cn - c0],
                                            op=ALU.add)

            # ---- phase B: per-width H build + second layer ----
            # H[kt] = (Ae[kt] shifted + bias) + As[kt]   (stt, DVE/GpSimd)
            # relu merged over all kt per width (ACT, no bias needed)
            # L2: widths paired on PE col-groups 0 / 64; mask+b2s fold into
            # the PSUM->SBUF epilogue stt on DVE.
            pending = []  # epilogues delayed one pair so DVE never
                          # stalls behind the PE inside the j-pipeline
            def flush_epi():
                while pending:
                    j, pl2p, cg = pending.pop(0)
                    mkt = mkp.tile([NT, WC], F32, tag="mkt", name="mkt")
                    nc.sync.dma_start(out=mkt[:], in_=mk_d[j])
                    lg = lgp.tile([NT, WC], F32, tag="lg", name="lg")
                    nc.vector.tensor_tensor(
                        lg[:], pl2p[cg:cg + NT, :], mkt[:], op=ALU.add)
                    nc.sync.dma_start(out=lg_d[j], in_=lg[:])

            for jp in range(MAXW // 2):
                j0 = 2 * jp
                H = hp.tile([P, KT, 2, WC], F32, tag="H")
                for jj in (0, 1):
                    j = j0 + jj
                    # kt 0-4: DVE stt folds the relu bias into the add;
                    # their relu runs bias-free, merged, on ACT.
                    for kt in range(5):
                        nc.vector.scalar_tensor_tensor(
                            H[:, kt, jj, :], Ae[:, kt, j:j + WC],
                            ct[:, kt * MAXW + j:kt * MAXW + j + 1],
                            As[:, kt, :], op0=ALU.add, op1=ALU.add)
                    nc.scalar.activation(H[:, 0:5, jj, :], H[:, 0:5, jj, :],
                                         AF.Relu, bias=0.0, scale=1.0)
                    # kt 5-7: plain adds on GpSimd, biased relu on ACT.
                    nc.gpsimd.tensor_tensor(
                        H[:, 5:8, jj, :], Ae[:, 5:8, j:j + WC], As[:, 5:8, :],
                        op=ALU.add)
                    for kt in range(5, KT):
                        nc.scalar.activation(
                            H[:, kt, jj, :], H[:, kt, jj, :], AF.Relu,
                            bias=ct[:, kt * MAXW + j:kt * MAXW + j + 1],
                            scale=1.0)
                pl2 = psl.tile([102, WC], F32, tag="pl2")
                for kt in range(KT):
                    for jj, cg in ((0, 0), (1, 64)):
                        nc.tensor.matmul(
                            pl2[cg:cg + NT, :],
                            w2[:, kt * NT:(kt + 1) * NT], H[:, kt, jj, :],
                            start=(kt == 0), stop=(kt == KT - 1),
                            tile_position=(0, cg), skip_group_check=True)
                pending.append((j0, pl2, 0))
                pending.append((j0 + 1, pl2, 64))
                if jp > 0 or True:
                    pass
                if jp >= 1:
                    # flush the PREVIOUS pair's epilogues now; DVE has the
                    # current pair's H work queued ahead of them
                    prev = pending[:-2]
                    pending[:] = pending[-2:]
                    for j, pl2p, cg in prev:
                        mkt = mkp.tile([NT, WC], F32, tag="mkt", name="mkt")
                        nc.sync.dma_start(out=mkt[:], in_=mk_d[j])
                        lg = lgp.tile([NT, WC], F32, tag="lg", name="lg")
                        nc.vector.tensor_tensor(
                            lg[:], pl2p[cg:cg + NT, :], mkt[:], op=ALU.add)
                        nc.sync.dma_start(out=lg_d[j], in_=lg[:])
            flush_epi()

    nc.compile()
    return nc


def _split_f16(x):
    hi = x.astype(np.float16)
    lo = ((x - hi.astype(np.float32)) * np.float32(2.0 ** 11)).astype(np.float16)
    return hi, lo


def _prep_inputs(encoded_doc, sent_map, span_width_emb, span_width_prior_emb,
                 doc_type_emb, W1, b1, W2, b2, Wp1, bp1, Wp2, bp2, doc_type):
    """Host-side sharding + weight re-layout (fp32 / fp16 hi-lo)."""
    E = np.ascontiguousarray(np.asarray(encoded_doc, np.float32))
    sm = np.asarray(sent_map).astype(np.int64)
    W1 = np.asarray(W1, np.float32)
    b1 = np.asarray(b1, np.float32)
    W2f = np.asarray(W2, np.float32)
    b2f = np.asarray(b2, np.float32)
    swe = np.asarray(span_width_emb, np.float32)
    swpe = np.asarray(span_width_prior_emb, np.float32)
    dte = np.asarray(doc_type_emb, np.float32)
    dt = int(np.asarray(doc_type))

    # E^T, padded so every core can slice [1024, WH]
    ETp = np.zeros((HS, NCORES * WC + (WH - WC)), np.float32)
    ETp[:, :NW] = E.T
    EThi, ETlo = _split_f16(ETp)

    # W1 endpoint halves -> [MT, P, KT*P] tiles; cols padded 1000->1024
    W1cat = np.zeros((HS, 2 * MLPP), np.float32)
    W1cat[:, 0:MLP] = W1[0:HS]
    W1cat[:, MLPP:MLPP + MLP] = W1[HS:2 * HS]
    w1t = W1cat.reshape(KT, P, MT, P).transpose(2, 1, 0, 3).reshape(MT, P, KT * P)
    w1h, w1l = _split_f16(np.ascontiguousarray(w1t))

    # W2 padded -> [P, KT*NT]
    W2pad = np.zeros((MLPP, NT), np.float32)
    W2pad[:MLP] = W2f
    w2t = np.ascontiguousarray(
        W2pad.reshape(KT, P, NT).transpose(1, 0, 2).reshape(P, KT * NT))

    # relu bias table: Aw[j] + dt_emb contribution + b1  -> [P, KT*MAXW]
    Aw = swe @ W1[2 * HS:2 * HS + EMB]                      # [20, 1000]
    dvec = dte[dt] @ W1[2 * HS + EMB:2 * HS + 2 * EMB]      # [1000]
    cpad = np.zeros((MAXW, MLPP), np.float32)
    cpad[:, :MLP] = Aw + dvec[None, :] + b1[None, :]
    ctab = np.ascontiguousarray(
        cpad.T.reshape(KT, P, MAXW).transpose(1, 0, 2).reshape(P, KT * MAXW))

    # width-prior scores + type bias -> [NT, MAXW]
    hw = np.maximum(swpe @ np.asarray(Wp1, np.float32) + np.asarray(bp1, np.float32), 0.0)
    ws = (hw @ np.asarray(Wp2, np.float32) + np.asarray(bp2, np.float32))[:, 0]
    b2s = np.ascontiguousarray(b2f[:, None] + ws[None, :])

    # per-core validity masks (additive 0 / NEG), replicated over types:
    # [NT, MAXW*WC], j-major
    in_maps = []
    for c in range(NCORES):
        w = c * WC + np.arange(WC)
        j = np.arange(MAXW)
        ends = w[:, None] + j[None, :]
        corr = np.minimum(ends, NW - 1)
        valid = (ends < NW) & (sm[w][:, None] == sm[corr])
        msk = np.where(valid, 0.0, np.float32(NEG)).astype(np.float32)
        mskr = np.ascontiguousarray(
            msk.T.reshape(MAXW, 1, WC) + b2s.T.reshape(MAXW, NT, 1))
        ehc = np.ascontiguousarray(
            EThi[:, c * WC:c * WC + WH].reshape(KT, P, WH))
        elc = np.ascontiguousarray(
            ETlo[:, c * WC:c * WC + WH].reshape(KT, P, WH))
        in_maps.append({
            "eh": ehc, "el": elc, "wh": w1h, "wl": w1l,
            "w2": w2t, "ct": ctab, "mskr": mskr,
        })
    return in_maps


def _topk_stable(flat, k):
    """Exact jax.lax.top_k: descending, ties broken by lower index."""
    kth = np.partition(flat, len(flat) - k)[len(flat) - k]
    cand = np.nonzero(flat >= kth)[0]
    order = np.lexsort((cand, -flat[cand]))[:k]
    idx = cand[order]
    return flat[idx], idx


def kernel(encoded_doc, sent_map, span_width_emb, span_width_prior_emb,
           doc_type_emb, W1, b1, W2, b2, Wp1, bp1, Wp2, bp2, doc_type, k,
           _return_results=False):
    if "nc" not in _CACHE:
        _CACHE["nc"] = _build_nc()
    nc = _CACHE["nc"]

    in_maps = _prep_inputs(encoded_doc, sent_map, span_width_emb,
                           span_width_prior_emb, doc_type_emb, W1, b1, W2, b2,
                           Wp1, bp1, Wp2, bp2, doc_type)
    res = run_bass_kernel_spmd(nc, in_maps, list(range(NCORES)))
    _CACHE["last_res"] = res

    # gather: per-core lg [MAXW, NT, WC] -> [w, j, t]
    logits = np.concatenate(
        [res.results[c]["lg"].transpose(2, 0, 1) for c in range(NCORES)], axis=0)
    flat = np.ascontiguousarray(logits).reshape(-1)

    kk = int(np.asarray(k))
    scores, fidx = _topk_stable(flat, kk)
    cand = (fidx // NT).astype(np.int64)
    starts = (cand // MAXW).astype(np.int32)
    width = (cand % MAXW).astype(np.int32)
    ends = np.minimum(starts + width, NW - 1).astype(np.int32)
    types = (fidx % NT).astype(np.int32)
    mask = np.zeros(NW * MAXW * NT, np.float32)
    mask[cand] = 1.0
    out = (starts, ends, scores.astype(np.float32), types, mask)
    if _return_results:
        return out, res
    return out


# revision 15
# speedup vs baseline: 1.1814x; 1.1814x over previous
"""Trainium2 Bass kernel for the span-scoring + top-k module.

Problem (see reference): score all (word, width) candidate spans of a
4096-word document with a 1-hidden-layer MLP over concatenated endpoint
embeddings + width embedding + doc-type embedding, add a width-prior
score, mask invalid spans, then take the global top-k over the flattened
(span, type) score matrix.

Key algebraic restructuring (exact up to fp32 rounding):
  span_embs @ W1 decomposes into
      As[w] + Ae[w+j] + (Aw[j] + dt + b1)
  where As = E @ W1[:1024], Ae = E @ W1[1024:2048] are per-WORD matmuls
  (20x fewer FLOPs than the naive per-span matmul), and the width/doc
  contributions are tiny per-width bias vectors.  For a fixed width j the
  end-gather Ae[w+j] is just a shifted slice - no gathers anywhere.

Sharding: words are split 8 ways (512 words/core + 19-word halo for the
end embeddings).  Weights are replicated.  Each core computes its
[512 words x 20 widths x 38 types] logit block; the host merges the
per-shard results and does the exact top-k (selection on 9.8K censored
candidates is host-side bookkeeping; all heavy compute is on-device).

Device layout per core (all fp32):
  ET   [8kt][128, 544]  E^T shard slice (host-transposed, zero-padded)
  W1 m-tiles streamed   [128, 8kt*128] (host-tiled so lhsT slices are
                        contiguous), m 0-7 -> As, 8-15 -> Ae
  As   [128, 8, 512], Ae [128, 8, 544]   (hidden on partitions)
  H_j = relu(As + Ae[:, :, j:j+512] + ctab[:, kt, j])   (DVE add + ACT relu)
  L_j = W2^T @ H_j  (+ validity mask via a K=1 ones-matmul into the same
        PSUM accumulation, + per-type bias b2s[:, j] on the PSUM->SBUF copy)
  out  lg [20, 38, 512] -> HBM
"""

import numpy as np

import concourse.bacc as bacc
import concourse.bass as bass
import concourse.mybir as mybir
import concourse.tile as tile
from concourse.bass_utils import run_bass_kernel_spmd

F32 = mybir.dt.float32
F16 = mybir.dt.float16
AF = mybir.ActivationFunctionType
ALU = mybir.AluOpType

NCORES = 8
NW = 4096          # num words
HS = 1024          # hidden size of encoder states
MAXW = 20          # max span width
EMB = 20
MLP = 1000         # mlp hidden
MLPP = 1024        # padded mlp hidden
NT = 38            # n types
NEG = -1e10
WC = 512           # words per core
WH = 544           # words + halo pad (>= 512 + 19, mult of 32)
KT = 8             # k tiles over HS
MT = 16            # m tiles over 2*MLPP
P = 128

_CACHE = {}

import os
FP32R_L1 = os.environ.get("FP32R_L1", "0") == "1"
FP32R_L2 = os.environ.get("FP32R_L2", "0") == "1"
F32R = mybir.dt.float32r


def _mm_l1(ap):
    return ap.bitcast(F32R) if FP32R_L1 else ap


def _mm_l2(ap):
    return ap.bitcast(F32R) if FP32R_L2 else ap


def _build_nc():
    """Build + compile the single-core SPMD Bass program.

    L1 runs in fp16 hi/lo split form: E = Eh + 2^-11*El, W1 = Wh + 2^-11*Wl
    (fp16 products are exact in the fp32 PSUM accumulator, so
    As = Eh@Wh + 2^-11*(Eh@Wl + El@Wh) is MORE accurate than a plain fp32
    matmul and streams 3 cycles/row instead of fp32's 4).
    """
    nc = bacc.Bacc(None, target_bir_lowering=False)

    eh_d = nc.dram_tensor("eh", [KT, P, WH], F16, kind="ExternalInput")
    el_d = nc.dram_tensor("el", [KT, P, WH], F16, kind="ExternalInput")
    wh_d = nc.dram_tensor("wh", [MT, P, KT * P], F16, kind="ExternalInput")
    wl_d = nc.dram_tensor("wl", [MT, P, KT * P], F16, kind="ExternalInput")
    w2_d = nc.dram_tensor("w2", [P, KT * NT], F32, kind="ExternalInput")
    ct_d = nc.dram_tensor("ct", [P, KT * MAXW], F32, kind="ExternalInput")
    mk_d = nc.dram_tensor("mskr", [MAXW, NT, WC], F32, kind="ExternalInput")
    lg_d = nc.dram_tensor("lg", [MAXW, NT, WC], F32, kind="ExternalOutput")

    LOSC = float(2.0 ** -11)

    with tile.TileContext(nc) as tc:
        with (
            tc.tile_pool(name="const", bufs=1) as constp,
            tc.tile_pool(name="acc", bufs=1) as accp,
            tc.tile_pool(name="w1p", bufs=3) as w1p,
            tc.tile_pool(name="mkp", bufs=4) as mkp,
            tc.tile_pool(name="hp", bufs=2) as hp,
            tc.tile_pool(name="lgp", bufs=4) as lgp,
            tc.tile_pool(name="psa", bufs=2, space="PSUM") as psa,
            tc.tile_pool(name="psb", bufs=1, space="PSUM") as psb,
            tc.tile_pool(name="psl", bufs=3, space="PSUM") as psl,
        ):
            # ---- constants / persistent tensors ----
            # first m-pair's weights go first so the PE can start ASAP
            w1h0 = w1p.tile([P, KT * P], F16, tag="w1h")
            w1l0 = w1p.tile([P, KT * P], F16, tag="w1l")
            nc.sync.dma_start(out=w1h0[:], in_=wh_d[0])
            nc.sync.dma_start(out=w1l0[:], in_=wl_d[0])
            eh = [constp.tile([P, WH], F16, tag=f"eh{kt}", name=f"eh{kt}")
                  for kt in range(KT)]
            el = [constp.tile([P, WH], F16, tag=f"el{kt}", name=f"el{kt}")
                  for kt in range(KT)]
            for kt in range(KT):
                nc.sync.dma_start(out=eh[kt][:], in_=eh_d[kt])
                nc.sync.dma_start(out=el[kt][:], in_=el_d[kt])
            w2 = constp.tile([P, KT * NT], F32)
            nc.sync.dma_start(out=w2[:], in_=w2_d[:])
            ct = constp.tile([P, KT * MAXW], F32)
            nc.sync.dma_start(out=ct[:], in_=ct_d[:])

            As = accp.tile([P, KT, WC], F32)
            Ae = accp.tile([P, KT, WH], F32)

            # pre-built H tiles for j-pairs 0 and 1 (filled during phase A)
            Hpre = [accp.tile([P, KT, 2, WC], F32, tag="Hp0", name="Hp0")]

            # ---- phase A: As/Ae = E @ W1{s,e}, fp16 hi/lo split ----
            # m-order pairs (kt_h, kt_h+8) so each hidden k-tile of As AND Ae
            # completes early, letting phase-B H adds overlap the tail.
            for mi in range(MT):
                kt_h, half = mi // 2, mi % 2
                m = kt_h + 8 * half  # 0,8,1,9,...  (m<8 -> As, m>=8 -> Ae)
                if mi == 0:
                    w1h, w1l = w1h0, w1l0
                else:
                    w1h = w1p.tile([P, KT * P], F16, tag="w1h")
                    w1l = w1p.tile([P, KT * P], F16, tag="w1l")
                    nc.sync.dma_start(out=w1h[:], in_=wh_d[m])
                    nc.sync.dma_start(out=w1l[:], in_=wl_d[m])
                dst, ncols = (As, WC) if m < KT else (Ae, WH)
                for c0, cn, pool, tagsuf in ((0, WC, psa, "a"), (WC, WH, psb, "b")):
                    if c0 >= ncols:
                        continue
                    if tagsuf == "b":
                        pb2 = pool.tile([P, 2 * (cn - c0)], F32, tag="phlb",
                                        name="phlb")
                        ph = pb2[:, 0:cn - c0]
                        pl = pb2[:, cn - c0:]
                    else:
                        ph = pool.tile([P, cn - c0], F32, tag="ph" + tagsuf,
                                       name="ph" + tagsuf)
                        pl = pool.tile([P, cn - c0], F32, tag="pl" + tagsuf,
                                       name="pl" + tagsuf)
                    for kt in range(KT):
                        nc.tensor.matmul(
                            ph[:], w1h[:, kt * P:(kt + 1) * P],
                            eh[kt][:, c0:cn],
                            start=(kt == 0), stop=(kt == KT - 1))
                    for kt in range(KT):
                        nc.tensor.matmul(
                            pl[:], w1l[:, kt * P:(kt + 1) * P],
                            eh[kt][:, c0:cn],
                            start=(kt == 0), stop=False)
                        nc.tensor.matmul(
                            pl[:], w1h[:, kt * P:(kt + 1) * P],
                            el[kt][:, c0:cn],
                            start=False, stop=(kt == KT - 1))
                    out_ap = dst[:, kt_h, c0:cn]
                    tmp = w1p.tile([P, WC], F32, tag="cmb", name="cmb")
                    nc.scalar.activation(tmp[:, 0:cn - c0], pl[:], AF.Copy,
                                         bias=0.0, scale=LOSC)
                    nc.vector.tensor_tensor(out_ap, ph[:], tmp[:, 0:cn - c0],
                                            op=ALU.add)
                if half == 1:
                    # As[kt_h] and Ae[kt_h] are now both final: pre-build the
                    # H slices of j-pairs 0/1 while the PE continues phase A.
                    for pp in range(1):
                        for jj in (0, 1):
                            j = 2 * pp + jj
                            nc.vector.scalar_tensor_tensor(
                                Hpre[pp][:, kt_h, jj, :],
                                Ae[:, kt_h, j:j + WC],
                                ct[:, kt_h * MAXW + j:kt_h * MAXW + j + 1],
                                As[:, kt_h, :], op0=ALU.add, op1=ALU.add)

            # ---- phase B: per-width H build + second layer ----
            # H[kt] = (Ae[kt] shifted + bias) + As[kt]   (stt, DVE/GpSimd)
            # relu merged over all kt per width (ACT, no bias needed)
            # L2: widths paired on PE col-groups 0 / 64; mask+b2s fold into
            # the PSUM->SBUF epilogue stt on DVE.
            pending = []  # epilogues delayed one pair so DVE never
                          # stalls behind the PE inside the j-pipeline
            def flush_epi():
                while pending:
                    j, pl2p, cg = pending.pop(0)
                    mkt = mkp.tile([NT, WC], F32, tag="mkt", name="mkt")
                    nc.sync.dma_start(out=mkt[:], in_=mk_d[j])
                    lg = lgp.tile([NT, WC], F32, tag="lg", name="lg")
                    nc.vector.tensor_tensor(
                        lg[:], pl2p[cg:cg + NT, :], mkt[:], op=ALU.add)
                    nc.sync.dma_start(out=lg_d[j], in_=lg[:])

            for jp in range(MAXW // 2):
                j0 = 2 * jp
                H = (hp.tile([P, KT, 2, WC], F32, tag="H", name="H")
                     if jp >= 1 else None)
                if jp >= 1:
                    for jj in (0, 1):
                        j = j0 + jj
                        nc.vector.tensor_tensor(
                            H[:, :, jj, :], Ae[:, :, j:j + WC], As[:, :, :],
                            op=ALU.add)
                        for kt in range(KT):
                            nc.scalar.activation(
                                H[:, kt, jj, :], H[:, kt, jj, :], AF.Relu,
                                bias=ct[:, kt * MAXW + j:kt * MAXW + j + 1],
                                scale=1.0)
                else:
                    # H for pair 0 was filled during phase A; just relu.
                    H = Hpre[jp]
                    for jj in (0, 1):
                        nc.scalar.activation(
                            H[:, :, jj, :], H[:, :, jj, :], AF.Relu,
                            bias=0.0, scale=1.0)
                pl2 = psl.tile([102, WC], F32, tag="pl2")
                for kt in range(KT):
                    for jj, cg in ((0, 0), (1, 64)):
                        nc.tensor.matmul(
                            pl2[cg:cg + NT, :],
                            w2[:, kt * NT:(kt + 1) * NT], H[:, kt, jj, :],
                            start=(kt == 0), stop=(kt == KT - 1),
                            tile_position=(0, cg), skip_group_check=True)
                pending.append((j0, pl2, 0))
                pending.append((j0 + 1, pl2, 64))
                if jp > 0 or True:
                    pass
                if jp >= 1:
                    # flush the PREVIOUS pair's epilogues now; DVE has the
                    # current pair's H work queued ahead of them
                    prev = pending[:-2]
                    pending[:] = pending[-2:]
                    for j, pl2p, cg in prev:
                        mkt = mkp.tile([NT, WC], F32, tag="mkt", name="mkt")
                        nc.sync.dma_start(out=mkt[:], in_=mk_d[j])
                        lg = lgp.tile([NT, WC], F32, tag="lg", name="lg")
                        nc.vector.tensor_tensor(
                            lg[:], pl2p[cg:cg + NT, :], mkt[:], op=ALU.add)
                        nc.sync.dma_start(out=lg_d[j], in_=lg[:])
            flush_epi()

    nc.compile()
    return nc


def _split_f16(x):
    hi = x.astype(np.float16)
    lo = ((x - hi.astype(np.float32)) * np.float32(2.0 ** 11)).astype(np.float16)
    return hi, lo


def _prep_inputs(encoded_doc, sent_map, span_width_emb, span_width_prior_emb,
                 doc_type_emb, W1, b1, W2, b2, Wp1, bp1, Wp2, bp2, doc_type):
    """Host-side sharding + weight re-layout (fp32 / fp16 hi-lo)."""
    E = np.ascontiguousarray(np.asarray(encoded_doc, np.float32))
    sm = np.asarray(sent_map).astype(np.int64)
    W1 = np.asarray(W1, np.float32)
    b1 = np.asarray(b1, np.float32)
    W2f = np.asarray(W2, np.float32)
    b2f = np.asarray(b2, np.float32)
    swe = np.asarray(span_width_emb, np.float32)
    swpe = np.asarray(span_width_prior_emb, np.float32)
    dte = np.asarray(doc_type_emb, np.float32)
    dt = int(np.asarray(doc_type))

    # E^T, padded so every core can slice [1024, WH]
    ETp = np.zeros((HS, NCORES * WC + (WH - WC)), np.float32)
    ETp[:, :NW] = E.T
    EThi, ETlo = _split_f16(ETp)

    # W1 endpoint halves -> [MT, P, KT*P] tiles; cols padded 1000->1024
    W1cat = np.zeros((HS, 2 * MLPP), np.float32)
    W1cat[:, 0:MLP] = W1[0:HS]
    W1cat[:, MLPP:MLPP + MLP] = W1[HS:2 * HS]
    w1t = W1cat.reshape(KT, P, MT, P).transpose(2, 1, 0, 3).reshape(MT, P, KT * P)
    w1h, w1l = _split_f16(np.ascontiguousarray(w1t))

    # W2 padded -> [P, KT*NT]
    W2pad = np.zeros((MLPP, NT), np.float32)
    W2pad[:MLP] = W2f
    w2t = np.ascontiguousarray(
        W2pad.reshape(KT, P, NT).transpose(1, 0, 2).reshape(P, KT * NT))

    # relu bias table: Aw[j] + dt_emb contribution + b1  -> [P, KT*MAXW]
    Aw = swe @ W1[2 * HS:2 * HS + EMB]                      # [20, 1000]
    dvec = dte[dt] @ W1[2 * HS + EMB:2 * HS + 2 * EMB]      # [1000]
    cpad = np.zeros((MAXW, MLPP), np.float32)
    cpad[:, :MLP] = Aw + dvec[None, :] + b1[None, :]
    ctab = np.ascontiguousarray(
        cpad.T.reshape(KT, P, MAXW).transpose(1, 0, 2).reshape(P, KT * MAXW))

    # width-prior scores + type bias -> [NT, MAXW]
    hw = np.maximum(swpe @ np.asarray(Wp1, np.float32) + np.asarray(bp1, np.float32), 0.0)
    ws = (hw @ np.asarray(Wp2, np.float32) + np.asarray(bp2, np.float32))[:, 0]
    b2s = np.ascontiguousarray(b2f[:, None] + ws[None, :])

    # per-core validity masks (additive 0 / NEG), replicated over types:
    # [NT, MAXW*WC], j-major
    in_maps = []
    for c in range(NCORES):
        w = c * WC + np.arange(WC)
        j = np.arange(MAXW)
        ends = w[:, None] + j[None, :]
        corr = np.minimum(ends, NW - 1)
        valid = (ends < NW) & (sm[w][:, None] == sm[corr])
        msk = np.where(valid, 0.0, np.float32(NEG)).astype(np.float32)
        mskr = np.ascontiguousarray(
            msk.T.reshape(MAXW, 1, WC) + b2s.T.reshape(MAXW, NT, 1))
        ehc = np.ascontiguousarray(
            EThi[:, c * WC:c * WC + WH].reshape(KT, P, WH))
        elc = np.ascontiguousarray(
            ETlo[:, c * WC:c * WC + WH].reshape(KT, P, WH))
        in_maps.append({
            "eh": ehc, "el": elc, "wh": w1h, "wl": w1l,
            "w2": w2t, "ct": ctab, "mskr": mskr,
        })
    return in_maps


def _topk_stable(flat, k):
    """Exact jax.lax.top_k: descending, ties broken by lower index."""
    kth = np.partition(flat, len(flat) - k)[len(flat) - k]
    cand = np.nonzero(flat >= kth)[0]
    order = np.lexsort((cand, -flat[cand]))[:k]
    idx = cand[order]
    return flat[idx], idx


def kernel(encoded_doc, sent_map, span_width_emb, span_width_prior_emb,
           doc_type_emb, W1, b1, W2, b2, Wp1, bp1, Wp2, bp2, doc_type, k,
           _return_results=False):
    if "nc" not in _CACHE:
        _CACHE["nc"] = _build_nc()
    nc = _CACHE["nc"]

    in_maps = _prep_inputs(encoded_doc, sent_map, span_width_emb,
                           span_width_prior_emb, doc_type_emb, W1, b1, W2, b2,
                           Wp1, bp1, Wp2, bp2, doc_type)
    res = run_bass_kernel_spmd(nc, in_maps, list(range(NCORES)))
    _CACHE["last_res"] = res

    # gather: per-core lg [MAXW, NT, WC] -> [w, j, t]
    logits = np.concatenate(
        [res.results[c]["lg"].transpose(2, 0, 1) for c in range(NCORES)], axis=0)
    flat = np.ascontiguousarray(logits).reshape(-1)

    kk = int(np.asarray(k))
    scores, fidx = _topk_stable(flat, kk)
    cand = (fidx // NT).astype(np.int64)
    starts = (cand // MAXW).astype(np.int32)
    width = (cand % MAXW).astype(np.int32)
    ends = np.minimum(starts + width, NW - 1).astype(np.int32)
    types = (fidx % NT).astype(np.int32)
    mask = np.zeros(NW * MAXW * NT, np.float32)
    mask[cand] = 1.0
    out = (starts, ends, scores.astype(np.float32), types, mask)
    if _return_results:
        return out, res
    return out
